# revision 4
# baseline (speedup 1.0000x reference)
"""Trainium2 Bass kernel for nn_DecoderGRU: 64 steps of a GRUCell where the
input at every step equals the hidden state, outputs stacked to [B, T, H].

Sharding: pure data-parallel over batch across 8 NeuronCores (64 rows/core);
GRU weights replicated per core. The recurrence is sequential in time.

Math exploited: because input == state, the r/z gates satisfy
    i_r + h_r = h @ (W_ir + W_hr).T   (same for z)
so the per-step matmul shrinks from [*,6H] to [*,4H]:
    g = h @ Wcat.T,  Wcat = [W_ir+W_hr; W_iz+W_hz; W_in; W_hn]  (4H x H)
    r = sigmoid(g_r + b_r);  z = sigmoid(g_z + b_z)
    n = tanh((g_in + b_in) + r * (g_hn + b_hn))
    h' = (1-z)*n + z*h = n*sigmoid(-(g_z+b_z)) + z*h

Per-core layout (per step):
  stationary lhsT = h.T as 8 chunks [128, 64] bf16 (kept as 2 tiles [128, 4*64])
  moving rhs      = Wcat.T in SBUF [128, 8*4096] bf16
  matmul out      = gates [64, 512] fp32 in PSUM, 8 chunks x 8 K-accumulations
  h'.T for the next step via 8 PE transposes ([64,128] -> [128,64]).
"""

import numpy as np
import ml_dtypes

import concourse.bass as bass
import concourse.mybir as mybir
from concourse import bacc
from concourse.tile import TileContext
from concourse.bass_utils import run_bass_kernel_spmd
from concourse.masks import make_identity

BATCH, H, T = 512, 1024, 64
NCORES = 8
B = BATCH // NCORES     # 64 batch rows per core
G = 4 * H               # 4096 gate columns: [r, z, in, hn] per 512-hidden-slice
KC = H // 128           # 8 contraction chunks
S = H // 512            # 2 hidden slices of 512
NG = 4                  # gate blocks per slice (r, z, in, hn)
FP32 = mybir.dt.float32
BF16 = mybir.dt.bfloat16
AF = mybir.ActivationFunctionType

_NC_CACHE = {}


def build_nc(n_steps: int = T) -> bass.Bass:
    # Bacc (not plain Bass): its compile() runs generate_event_semaphores,
    # which splits multi-waits — walrus codegen allows only 1 sync wait per
    # compute instruction on trn2.
    nc = bacc.Bacc()

    w_d = nc.dram_tensor("w", [KC, 128, G], BF16, kind="ExternalInput")
    bias_d = nc.dram_tensor("bias", [B, G], FP32, kind="ExternalInput")
    h0_d = nc.dram_tensor("h0", [B, H], FP32, kind="ExternalInput")
    h0t_d = nc.dram_tensor("h0t", [S, 128, NG * B], BF16, kind="ExternalInput")
    out_d = nc.dram_tensor("out", [B, n_steps, H], FP32, kind="ExternalOutput")

    with TileContext(nc) as tc:
        with (
            tc.tile_pool(name="const", bufs=1) as constp,
            tc.tile_pool(name="htp", bufs=3) as htp,
            tc.tile_pool(name="hbuf", bufs=3) as hbuf,
            tc.tile_pool(name="work", bufs=2) as work,
            tc.tile_pool(name="gpsum", bufs=6, space="PSUM") as gpsum,
            tc.tile_pool(name="tpsum", bufs=2, space="PSUM") as tpsum,
        ):
            # ---- constants ----
            w_sb = []
            for k in range(KC):
                wt = constp.tile([128, G], BF16, tag=f"w{k}", name=f"w_sb{k}")
                nc.sync.dma_start(wt, w_d[k])
                w_sb.append(wt)
            bias_sb = constp.tile([B, G], FP32, tag="bias", name="bias_sb")
            nc.sync.dma_start(bias_sb, bias_d[:])
            ident = constp.tile([B, B], FP32, tag="ident", name="ident")
            make_identity(nc, ident)

            # ---- initial state ----
            hT = []
            for s in range(S):
                t0 = htp.tile([128, NG * B], BF16, tag=f"hT{s}", name=f"hT_init{s}")
                nc.sync.dma_start(t0, h0t_d[s])
                hT.append(t0)
            h_cur = []
            for s in range(S):
                hc = hbuf.tile([B, 512], FP32, tag=f"h{s}", name=f"h_init{s}")
                nc.sync.dma_start(hc, h0_d[:, s * 512:(s + 1) * 512])
                h_cur.append(hc)

            def bias_sl(s, gi):
                c0 = (s * NG + gi) * 512
                return bias_sb[:, c0:c0 + 512]

            for t in range(n_steps):
                # ---- matmuls: gates for both slices ----
                ps = []  # ps[s][gi] psum tiles
                for s in range(S):
                    pss = []
                    for gi in range(NG):
                        c0 = (s * NG + gi) * 512
                        pt = gpsum.tile([B, 512], FP32, tag="gate",
                                        name=f"g{t}_{s}_{gi}")
                        for k in range(KC):
                            nc.tensor.matmul(
                                pt,
                                hT[k // NG][:, (k % NG) * B:(k % NG + 1) * B],
                                w_sb[k][:, c0:c0 + 512],
                                start=(k == 0),
                                stop=(k == KC - 1),
                            )
                        pss.append(pt)
                    ps.append(pss)

                hT_new = [None, None]
                h_new = [None, None]
                for s in range(S):
                    pr, pz, pin, phn = ps[s]
                    # ---- elementwise ----
                    tr = work.tile([B, 512], FP32, tag="tr", name=f"tr{t}_{s}")
                    nc.vector.tensor_add(tr, pr, bias_sl(s, 0))
                    tz = work.tile([B, 512], FP32, tag="tz", name=f"tz{t}_{s}")
                    nc.vector.tensor_add(tz, pz, bias_sl(s, 1))
                    t1 = work.tile([B, 512], FP32, tag="t1", name=f"t1{t}_{s}")
                    nc.vector.tensor_add(t1, phn, bias_sl(s, 3))
                    pinb = work.tile([B, 512], FP32, tag="pinb", name=f"pinb{t}_{s}")
                    nc.vector.tensor_add(pinb, pin, bias_sl(s, 2))
                    r = work.tile([B, 512], FP32, tag="r", name=f"r{t}_{s}")
                    nc.scalar.activation(r, tr, AF.Sigmoid)
                    z = work.tile([B, 512], FP32, tag="z", name=f"z{t}_{s}")
                    nc.scalar.activation(z, tz, AF.Sigmoid)
                    z1 = work.tile([B, 512], FP32, tag="z1", name=f"z1{t}_{s}")
                    nc.scalar.activation(z1, tz, AF.Sigmoid, scale=-1.0)
                    zh = work.tile([B, 512], FP32, tag="zh", name=f"zh{t}_{s}")
                    nc.vector.tensor_mul(zh, z, h_cur[s])
                    t2 = work.tile([B, 512], FP32, tag="t2", name=f"t2{t}_{s}")
                    nc.vector.tensor_mul(t2, r, t1)
                    t3 = work.tile([B, 512], FP32, tag="t3", name=f"t3{t}_{s}")
                    nc.vector.tensor_add(t3, t2, pinb)
                    n = work.tile([B, 512], FP32, tag="n", name=f"n{t}_{s}")
                    nc.scalar.activation(n, t3, AF.Tanh)
                    nz1 = work.tile([B, 512], FP32, tag="nz1", name=f"nz1{t}_{s}")
                    nc.vector.tensor_mul(nz1, n, z1)
                    hn = hbuf.tile([B, 512], FP32, tag=f"h{s}", name=f"h{t}_{s}")
                    nc.vector.tensor_add(hn, nz1, zh)
                    nc.sync.dma_start(out_d[:, t, s * 512:(s + 1) * 512], hn)
                    h_new[s] = hn

                    # ---- transpose h' slice -> next-step stationary ----
                    tp = tpsum.tile([128, NG * B], FP32, tag="tp",
                                    name=f"tp{t}_{s}")
                    for kk in range(NG):
                        nc.tensor.transpose(
                            tp[:, kk * B:(kk + 1) * B],
                            hn[:, kk * 128:(kk + 1) * 128],
                            ident,
                        )
                    ht = htp.tile([128, NG * B], BF16, tag=f"hT{s}",
                                  name=f"hTn{t}_{s}")
                    nc.scalar.copy(ht, tp)
                    hT_new[s] = ht

                hT = hT_new
                h_cur = h_new

    nc.compile()
    return nc


def prepare_core_inputs(hidden, weight_ih, weight_hh, bias_ih, bias_hh):
    """Host-side preprocessing + batch sharding -> per-core input maps."""
    f32 = np.float32
    Wi = np.asarray(weight_ih, f32)
    Wh = np.asarray(weight_hh, f32)
    bi = np.asarray(bias_ih, f32)
    bh = np.asarray(bias_hh, f32)
    hidden = np.asarray(hidden, f32)

    # Wg[gi] : [H, H] with rows = gate output unit, cols = contraction k
    Wg = np.stack([
        Wi[0:H] + Wh[0:H],          # r
        Wi[H:2 * H] + Wh[H:2 * H],  # z
        Wi[2 * H:3 * H],            # in
        Wh[2 * H:3 * H],            # hn
    ])  # [4, H, H]
    # -> wdram[kc, p, (s*4+gi)*512 + j] = Wg[gi, s*512+j, kc*128+p]
    wdram = (
        Wg.reshape(NG, S, 512, KC, 128)      # [gi, s, j, kc, p]
        .transpose(3, 4, 1, 0, 2)            # [kc, p, s, gi, j]
        .reshape(KC, 128, G)
        .astype(ml_dtypes.bfloat16)
    )

    bias_g = np.stack([
        bi[0:H] + bh[0:H],
        bi[H:2 * H] + bh[H:2 * H],
        bi[2 * H:3 * H],
        bh[2 * H:3 * H],
    ])  # [4, H]
    bias_cols = bias_g.reshape(NG, S, 512).transpose(1, 0, 2).reshape(G)
    bias_t = np.ascontiguousarray(np.broadcast_to(bias_cols, (B, G)), dtype=f32)

    in_maps = []
    for c in range(NCORES):
        h0 = np.ascontiguousarray(hidden[c * B:(c + 1) * B])  # [B, H]
        h0t = (
            h0.reshape(B, S, NG, 128)   # [b, s, kk, p]
            .transpose(1, 3, 2, 0)      # [s, p, kk, b]
            .reshape(S, 128, NG * B)
            .astype(ml_dtypes.bfloat16)
        )
        in_maps.append({"w": wdram, "bias": bias_t, "h0": h0, "h0t": h0t})
    return in_maps


def _get_nc():
    if "nc" not in _NC_CACHE:
        _NC_CACHE["nc"] = build_nc(T)
    return _NC_CACHE["nc"]


def run(inputs: dict, trace: bool = False):
    """Returns (output [BATCH, T, H] fp32, BassKernelResults)."""
    in_maps = prepare_core_inputs(**inputs)
    res = run_bass_kernel_spmd(_get_nc(), in_maps, list(range(NCORES)),
                               trace=trace)
    out = np.concatenate([res.results[c]["out"] for c in range(NCORES)], axis=0)
    return out.astype(np.float32), res


def kernel(**inputs) -> np.ndarray:
    out, _ = run(inputs)
    return out


# revision 5
# speedup vs baseline: 1.1760x; 1.1760x over previous
"""Trainium2 Bass kernel for nn_DecoderGRU: 64 steps of a GRUCell where the
input at every step equals the hidden state, outputs stacked to [B, T, H].

Sharding: pure data-parallel over batch across 8 NeuronCores (64 rows/core);
GRU weights replicated per core. The recurrence is sequential in time.

Math exploited: because input == state, the r/z gates satisfy
    i_r + h_r = h @ (W_ir + W_hr).T   (same for z)
so the per-step matmul shrinks from [*,6H] to [*,4H]:
    g = h @ Wcat.T,  Wcat = [W_ir+W_hr; W_iz+W_hz; W_in; W_hn]  (4H x H)
    r = sigmoid(g_r + b_r);  z = sigmoid(g_z + b_z)
    n = tanh((g_in + b_in) + r * (g_hn + b_hn))
    h' = (1-z)*n + z*h = n*sigmoid(-(g_z+b_z)) + z*h

Layout (per core, per step): the per-core batch is only 64 rows, so a single
matmul would use half the PE array (M=64). Instead the two 512-wide hidden
slices are packed side by side in the array via PSUM column tiling:
  - psum gate tiles are [128, 512]: rows 0:64 = slice-0 batch, 64:128 = slice-1
  - each (gate, k-chunk) emits TWO matmuls sharing the stationary h.T chunk,
    one per output partition half (tile_position cols 0/64) -> they run
    concurrently in different column groups of the array
  - all elementwise runs fused [128, *] (both slices at once)
  - h'.T for the next step via 8 PE transposes ([64,128] -> [128,64]),
    row-tiled in pairs (slice-0 rows 0:64, slice-1 rows 64:128) to distinct
    PSUM banks so they run concurrently
"""

import numpy as np
import ml_dtypes

import concourse.bass as bass
import concourse.mybir as mybir
from concourse import bacc
from concourse.tile import TileContext
from concourse.bass_utils import run_bass_kernel_spmd

BATCH, H, T = 512, 1024, 64
NCORES = 8
B = BATCH // NCORES     # 64 batch rows per core
G = 4 * H               # 4096 gate columns: [r, z, in, hn] per 512-hidden-slice
KC = H // 128           # 8 contraction chunks
NG = 4                  # gate blocks (r, z, in, hn)
FP32 = mybir.dt.float32
BF16 = mybir.dt.bfloat16
AF = mybir.ActivationFunctionType

# k-chunk -> (hT tile index, column offset). hTa holds chunks {0,1,4,5} (the
# ones produced by the first column-half of h'), hTb holds {2,3,6,7}.
KC_TILE = {0: (0, 0), 1: (0, 64), 4: (0, 128), 5: (0, 192),
           2: (1, 0), 3: (1, 64), 6: (1, 128), 7: (1, 192)}
KC_ORDER = [0, 1, 4, 5, 2, 3, 6, 7]

_NC_CACHE = {}


def build_nc(n_steps: int = T) -> bass.Bass:
    # Bacc (not plain Bass): its compile() runs generate_event_semaphores,
    # which splits multi-waits — walrus codegen allows only 1 sync wait per
    # compute instruction on trn2.
    nc = bacc.Bacc()

    w_d = nc.dram_tensor("w", [KC, 128, G], BF16, kind="ExternalInput")
    bias_d = nc.dram_tensor("bias", [128, NG * 512], FP32, kind="ExternalInput")
    h0_d = nc.dram_tensor("h0", [128, 512], FP32, kind="ExternalInput")
    h0t_d = nc.dram_tensor("h0t", [2, 128, NG * B], BF16, kind="ExternalInput")
    id_d = nc.dram_tensor("ident", [128, B], FP32, kind="ExternalInput")
    out_d = nc.dram_tensor("out", [B, n_steps, H], FP32, kind="ExternalOutput")

    with TileContext(nc) as tc:
        with (
            tc.tile_pool(name="const", bufs=1) as constp,
            tc.tile_pool(name="htp", bufs=3) as htp,
            tc.tile_pool(name="hbuf", bufs=3) as hbuf,
            tc.tile_pool(name="work", bufs=2) as work,
            tc.tile_pool(name="gpsum", bufs=5, space="PSUM") as gpsum,
            tc.tile_pool(name="tpsum", bufs=1, space="PSUM") as tpsum,
        ):
            # ---- constants ----
            w_sb = []
            for k in range(KC):
                wt = constp.tile([128, G], BF16, tag=f"w{k}", name=f"w_sb{k}")
                nc.sync.dma_start(wt, w_d[k])
                w_sb.append(wt)
            bias_sb = constp.tile([128, NG * 512], FP32, tag="bias", name="bias_sb")
            nc.sync.dma_start(bias_sb, bias_d[:])
            ident = constp.tile([128, B], FP32, tag="ident", name="ident")
            nc.sync.dma_start(ident, id_d[:])

            # ---- initial state ----
            hT = []
            for a in range(2):
                t0 = htp.tile([128, NG * B], BF16, tag=f"hT{a}", name=f"hT_init{a}")
                nc.sync.dma_start(t0, h0t_d[a])
                hT.append(t0)
            h_cur = hbuf.tile([128, 512], FP32, tag="h", name="h_init")
            nc.sync.dma_start(h_cur, h0_d[:])

            def bias_sl(gi, ch):
                c0 = gi * 512 + ch * 256
                return bias_sb[:, c0:c0 + 256]

            for t in range(n_steps):
                # ---- matmuls: 4 gate tiles, both slices col-tiled ----
                pg = []
                for gi in range(NG):
                    pt = gpsum.tile([128, 512], FP32, tag="gate",
                                    name=f"g{t}_{gi}")
                    for i, kc in enumerate(KC_ORDER):
                        a, c0 = KC_TILE[kc]
                        lh = hT[a][:, c0:c0 + B]
                        for s in range(2):
                            nc.tensor.matmul(
                                pt[s * B:(s + 1) * B, :],
                                lh,
                                w_sb[kc][:, (s * NG + gi) * 512:(s * NG + gi + 1) * 512],
                                start=(i == 0),
                                stop=(i == KC - 1),
                                skip_group_check=True,
                            )
                    pg.append(pt)

                hnew = hbuf.tile([128, 512], FP32, tag="h", name=f"h{t}")
                hT_new = [
                    htp.tile([128, NG * B], BF16, tag=f"hT{a}", name=f"hTn{t}_{a}")
                    for a in range(2)
                ]
                tpa = tpsum.tile([128, 256], FP32, tag="tpa", name=f"tpa{t}")
                tpb = tpsum.tile([128, 256], FP32, tag="tpb", name=f"tpb{t}")

                for ch in range(2):  # column half of the hidden slice
                    cs = slice(ch * 256, (ch + 1) * 256)
                    pr, pz, pin, phn = (p[:, cs] for p in pg)
                    # ---- elementwise (both slices fused, [128, 256]) ----
                    tr = work.tile([128, 256], FP32, tag="tr", name=f"tr{t}_{ch}")
                    nc.vector.tensor_add(tr, pr, bias_sl(0, ch))
                    tz = work.tile([128, 256], FP32, tag="tz", name=f"tz{t}_{ch}")
                    nc.vector.tensor_add(tz, pz, bias_sl(1, ch))
                    t1 = work.tile([128, 256], FP32, tag="t1", name=f"t1{t}_{ch}")
                    nc.vector.tensor_add(t1, phn, bias_sl(3, ch))
                    pinb = work.tile([128, 256], FP32, tag="pinb", name=f"pinb{t}_{ch}")
                    nc.vector.tensor_add(pinb, pin, bias_sl(2, ch))
                    r = work.tile([128, 256], FP32, tag="r", name=f"r{t}_{ch}")
                    nc.scalar.activation(r, tr, AF.Sigmoid)
                    z = work.tile([128, 256], FP32, tag="z", name=f"z{t}_{ch}")
                    nc.scalar.activation(z, tz, AF.Sigmoid)
                    z1 = work.tile([128, 256], FP32, tag="z1", name=f"z1{t}_{ch}")
                    nc.scalar.activation(z1, tz, AF.Sigmoid, scale=-1.0)
                    zh = work.tile([128, 256], FP32, tag="zh", name=f"zh{t}_{ch}")
                    nc.vector.tensor_mul(zh, z, h_cur[:, cs])
                    t2 = work.tile([128, 256], FP32, tag="t2", name=f"t2{t}_{ch}")
                    nc.vector.tensor_mul(t2, r, t1)
                    t3 = work.tile([128, 256], FP32, tag="t3", name=f"t3{t}_{ch}")
                    nc.vector.tensor_add(t3, t2, pinb)
                    n = work.tile([128, 256], FP32, tag="n", name=f"n{t}_{ch}")
                    nc.scalar.activation(n, t3, AF.Tanh)
                    nz1 = work.tile([128, 256], FP32, tag="nz1", name=f"nz1{t}_{ch}")
                    nc.vector.tensor_mul(nz1, n, z1)
                    nc.vector.tensor_add(hnew[:, cs], nz1, zh)

                    # ---- transposes for this column half ----
                    # chunks kc = s*4 + kk for kk in {2ch, 2ch+1}; pairs
                    # (s=0, s=1) go to distinct banks (tpa/tpb) and distinct
                    # row groups -> concurrent on the PE.
                    for kk in (2 * ch, 2 * ch + 1):
                        for s in range(2):
                            kc = s * NG + kk
                            tp = tpa if kc < 4 else tpb
                            nc.tensor.transpose(
                                tp[:, (kc % 4) * B:(kc % 4) * B + B],
                                hnew[s * B:(s + 1) * B, kk * 128:(kk + 1) * 128],
                                ident[s * B:(s + 1) * B, :],
                            )
                    # ---- store this column half of h' ----
                    for s in range(2):
                        nc.sync.dma_start(
                            out_d[:, t, s * 512 + ch * 256: s * 512 + ch * 256 + 256],
                            hnew[s * B:(s + 1) * B, cs],
                        )
                    # ---- pack transposed chunks into next-step hT tiles ----
                    # hTa cols: [kc0 | kc1 | kc4 | kc5], hTb: [kc2 | kc3 | kc6 | kc7]
                    a = ch  # ch=0 -> chunks {0,1,4,5} = hT tile 0
                    kk0 = 2 * ch
                    nc.scalar.copy(hT_new[a][:, 0:128], tpa[:, kk0 * B:(kk0 + 2) * B])
                    nc.scalar.copy(hT_new[a][:, 128:256], tpb[:, kk0 * B:(kk0 + 2) * B])

                hT = hT_new
                h_cur = hnew

    nc.compile()
    return nc


def prepare_core_inputs(hidden, weight_ih, weight_hh, bias_ih, bias_hh):
    """Host-side preprocessing + batch sharding -> per-core input maps."""
    f32 = np.float32
    Wi = np.asarray(weight_ih, f32)
    Wh = np.asarray(weight_hh, f32)
    bi = np.asarray(bias_ih, f32)
    bh = np.asarray(bias_hh, f32)
    hidden = np.asarray(hidden, f32)

    # Wg[gi] : [H, H] with rows = gate output unit, cols = contraction k
    Wg = np.stack([
        Wi[0:H] + Wh[0:H],          # r
        Wi[H:2 * H] + Wh[H:2 * H],  # z
        Wi[2 * H:3 * H],            # in
        Wh[2 * H:3 * H],            # hn
    ])  # [4, H, H]
    # -> wdram[kc, p, (s*4+gi)*512 + j] = Wg[gi, s*512+j, kc*128+p]
    wdram = (
        Wg.reshape(NG, 2, 512, KC, 128)      # [gi, s, j, kc, p]
        .transpose(3, 4, 1, 0, 2)            # [kc, p, s, gi, j]
        .reshape(KC, 128, G)
        .astype(ml_dtypes.bfloat16)
    )

    bias_g = np.stack([
        bi[0:H] + bh[0:H],
        bi[H:2 * H] + bh[H:2 * H],
        bi[2 * H:3 * H],
        bh[2 * H:3 * H],
    ])  # [4, H]
    # bias rows: 0:64 slice-0 values, 64:128 slice-1; cols gi*512 + j
    bias_rows = [bias_g[:, s * 512:(s + 1) * 512].reshape(NG * 512) for s in range(2)]
    bias_t = np.concatenate([
        np.broadcast_to(bias_rows[0], (B, NG * 512)),
        np.broadcast_to(bias_rows[1], (B, NG * 512)),
    ], axis=0).astype(f32)
    bias_t = np.ascontiguousarray(bias_t)

    ident = np.ascontiguousarray(np.tile(np.eye(B, dtype=f32), (2, 1)))

    perm = [[0, 1, 4, 5], [2, 3, 6, 7]]
    in_maps = []
    for c in range(NCORES):
        h0 = np.ascontiguousarray(hidden[c * B:(c + 1) * B])  # [B, H]
        h0_2 = np.concatenate([h0[:, :512], h0[:, 512:]], axis=0)  # [128, 512]
        hh = h0.reshape(B, KC, 128)  # [b, kc, p]
        h0t = np.stack([hh[:, perm[0], :], hh[:, perm[1], :]])  # [a, b, idx, p]
        h0t = np.ascontiguousarray(h0t.transpose(0, 3, 2, 1)   # [a, p, idx, b]
                                   .reshape(2, 128, NG * B)).astype(ml_dtypes.bfloat16)
        in_maps.append({"w": wdram, "bias": bias_t, "h0": h0_2, "h0t": h0t,
                        "ident": ident})
    return in_maps


def _get_nc():
    if "nc" not in _NC_CACHE:
        _NC_CACHE["nc"] = build_nc(T)
    return _NC_CACHE["nc"]


def run(inputs: dict, trace: bool = False):
    """Returns (output [BATCH, T, H] fp32, BassKernelResults)."""
    in_maps = prepare_core_inputs(**inputs)
    res = run_bass_kernel_spmd(_get_nc(), in_maps, list(range(NCORES)),
                               trace=trace)
    out = np.concatenate([res.results[c]["out"] for c in range(NCORES)], axis=0)
    return out.astype(np.float32), res


def kernel(**inputs) -> np.ndarray:
    out, _ = run(inputs)
    return out


# revision 8
# speedup vs baseline: 1.1843x; 1.0070x over previous
"""Trainium2 Bass kernel for nn_DecoderGRU: 64 steps of a GRUCell where the
input at every step equals the hidden state, outputs stacked to [B, T, H].

Sharding: pure data-parallel over batch across 8 NeuronCores (64 rows/core);
GRU weights replicated per core. The recurrence is sequential in time.

Math exploited: because input == state, the r/z gates satisfy
    i_r + h_r = h @ (W_ir + W_hr).T   (same for z)
so the per-step matmul shrinks from [*,6H] to [*,4H]:
    g = h @ Wcat.T,  Wcat = [W_ir+W_hr; W_iz+W_hz; W_in; W_hn]  (4H x H)
    r = sigmoid(g_r + b_r);  z = sigmoid(g_z + b_z)
    n = tanh((g_in + b_in) + r * (g_hn + b_hn))
    h' = (1-z)*n + z*h = n*sigmoid(-(g_z+b_z)) + z*h

Layout (per core, per step): the per-core batch is only 64 rows, so a single
matmul would use half the PE array (M=64). Instead the two 512-wide hidden
slices are packed side by side in the array via PSUM column tiling:
  - psum gate tiles are [128, 512]: rows 0:64 = slice-0 batch, 64:128 = slice-1
  - each (gate, k-chunk) emits TWO matmuls sharing the stationary h.T chunk,
    one per output partition half (tile_position cols 0/64) -> they run
    concurrently in different column groups of the array
  - all elementwise runs fused [128, *] (both slices at once)
  - h'.T for the next step via 8 PE transposes ([64,128] -> [128,64]),
    row-tiled in pairs (slice-0 rows 0:64, slice-1 rows 64:128) to distinct
    PSUM banks so they run concurrently
"""

import numpy as np
import ml_dtypes

import concourse.bass as bass
import concourse.mybir as mybir
from concourse import bacc
from concourse.tile import TileContext
from concourse.bass_utils import run_bass_kernel_spmd

BATCH, H, T = 512, 1024, 64
NCORES = 8
B = BATCH // NCORES     # 64 batch rows per core
G = 4 * H               # 4096 gate columns: [r, z, in, hn] per 512-hidden-slice
KC = H // 128           # 8 contraction chunks
NG = 4                  # gate blocks (r, z, in, hn)
FP32 = mybir.dt.float32
BF16 = mybir.dt.bfloat16
AF = mybir.ActivationFunctionType

# k-chunk -> (hT tile index, column offset). hTa holds chunks {0,1,4,5} (the
# ones produced by the first column-half of h'), hTb holds {2,3,6,7}.
KC_TILE = {0: (0, 0), 1: (0, 64), 4: (0, 128), 5: (0, 192),
           2: (1, 0), 3: (1, 64), 6: (1, 128), 7: (1, 192)}
KC_ORDER = [0, 1, 4, 5, 2, 3, 6, 7]

_NC_CACHE = {}


def build_nc(n_steps: int = T) -> bass.Bass:
    # Bacc (not plain Bass): its compile() runs generate_event_semaphores,
    # which splits multi-waits — walrus codegen allows only 1 sync wait per
    # compute instruction on trn2.
    nc = bacc.Bacc()

    w_d = nc.dram_tensor("w", [KC, 128, G], BF16, kind="ExternalInput")
    bias_d = nc.dram_tensor("bias", [128, NG * 512], FP32, kind="ExternalInput")
    h0_d = nc.dram_tensor("h0", [128, 512], FP32, kind="ExternalInput")
    h0t_d = nc.dram_tensor("h0t", [2, 128, NG * B], BF16, kind="ExternalInput")
    id_d = nc.dram_tensor("ident", [128, B], FP32, kind="ExternalInput")
    out_d = nc.dram_tensor("out", [B, n_steps, H], FP32, kind="ExternalOutput")

    with TileContext(nc) as tc:
        with (
            tc.tile_pool(name="const", bufs=1) as constp,
            tc.tile_pool(name="htp", bufs=3) as htp,
            tc.tile_pool(name="hbuf", bufs=3) as hbuf,
            tc.tile_pool(name="work", bufs=2) as work,
            tc.tile_pool(name="gpsum", bufs=5, space="PSUM") as gpsum,
            tc.tile_pool(name="tpsum", bufs=1, space="PSUM") as tpsum,
        ):
            # ---- constants ----
            w_sb = []
            for k in range(KC):
                wt = constp.tile([128, G], BF16, tag=f"w{k}", name=f"w_sb{k}")
                nc.sync.dma_start(wt, w_d[k])
                w_sb.append(wt)
            bias_sb = constp.tile([128, NG * 512], FP32, tag="bias", name="bias_sb")
            nc.sync.dma_start(bias_sb, bias_d[:])
            ident = constp.tile([128, B], FP32, tag="ident", name="ident")
            nc.sync.dma_start(ident, id_d[:])

            # ---- initial state ----
            hT = []
            for a in range(2):
                t0 = htp.tile([128, NG * B], BF16, tag=f"hT{a}", name=f"hT_init{a}")
                nc.sync.dma_start(t0, h0t_d[a])
                hT.append(t0)
            h_cur = hbuf.tile([128, 512], FP32, tag="h", name="h_init")
            nc.sync.dma_start(h_cur, h0_d[:])

            def bias_sl(gi, ch):
                c0 = gi * 512 + ch * 256
                return bias_sb[:, c0:c0 + 256]

            for t in range(n_steps):
                # ---- matmuls: 4 gate tiles, both slices col-tiled ----
                # Gate completion order [r, hn, in, z]: the elementwise tail
                # after the LAST gate is shortest when z lands last (only
                # sigmoid -> nz1 -> h' remains), so h'.T is ready sooner.
                pg = [None] * NG
                for gi in (0, 3, 2, 1):
                    pt = gpsum.tile([128, 512], FP32, tag="gate",
                                    name=f"g{t}_{gi}")
                    for i, kc in enumerate(KC_ORDER):
                        a, c0 = KC_TILE[kc]
                        lh = hT[a][:, c0:c0 + B]
                        for s in range(2):
                            nc.tensor.matmul(
                                pt[s * B:(s + 1) * B, :],
                                lh,
                                w_sb[kc][:, (s * NG + gi) * 512:(s * NG + gi + 1) * 512],
                                start=(i == 0),
                                stop=(i == KC - 1),
                                skip_group_check=True,
                            )
                    pg[gi] = pt

                hnew = hbuf.tile([128, 512], FP32, tag="h", name=f"h{t}")
                hT_new = [
                    htp.tile([128, NG * B], BF16, tag=f"hT{a}", name=f"hTn{t}_{a}")
                    for a in range(2)
                ]
                tpa = tpsum.tile([128, 256], FP32, tag="tpa", name=f"tpa{t}")
                tpb = tpsum.tile([128, 256], FP32, tag="tpb", name=f"tpb{t}")

                for ch in range(2):  # column half of the hidden slice
                    cs = slice(ch * 256, (ch + 1) * 256)
                    pr, pz, pin, phn = (p[:, cs] for p in pg)
                    # ---- elementwise (both slices fused, [128, 256]) ----
                    tr = work.tile([128, 256], FP32, tag="tr", name=f"tr{t}_{ch}")
                    nc.vector.tensor_add(tr, pr, bias_sl(0, ch))
                    t1 = work.tile([128, 256], FP32, tag="t1", name=f"t1{t}_{ch}")
                    nc.vector.tensor_add(t1, phn, bias_sl(3, ch))
                    pinb = work.tile([128, 256], FP32, tag="pinb", name=f"pinb{t}_{ch}")
                    nc.vector.tensor_add(pinb, pin, bias_sl(2, ch))
                    r = work.tile([128, 256], FP32, tag="r", name=f"r{t}_{ch}")
                    nc.scalar.activation(r, tr, AF.Sigmoid)
                    t2 = work.tile([128, 256], FP32, tag="t2", name=f"t2{t}_{ch}")
                    nc.vector.tensor_mul(t2, r, t1)
                    t3 = work.tile([128, 256], FP32, tag="t3", name=f"t3{t}_{ch}")
                    nc.vector.tensor_add(t3, t2, pinb)
                    n = work.tile([128, 256], FP32, tag="n", name=f"n{t}_{ch}")
                    nc.scalar.activation(n, t3, AF.Tanh)
                    tz = work.tile([128, 256], FP32, tag="tz", name=f"tz{t}_{ch}")
                    nc.vector.tensor_add(tz, pz, bias_sl(1, ch))
                    z1 = work.tile([128, 256], FP32, tag="z1", name=f"z1{t}_{ch}")
                    nc.scalar.activation(z1, tz, AF.Sigmoid, scale=-1.0)
                    z = work.tile([128, 256], FP32, tag="z", name=f"z{t}_{ch}")
                    nc.scalar.activation(z, tz, AF.Sigmoid)
                    nz1 = work.tile([128, 256], FP32, tag="nz1", name=f"nz1{t}_{ch}")
                    nc.vector.tensor_mul(nz1, n, z1)
                    zh = work.tile([128, 256], FP32, tag="zh", name=f"zh{t}_{ch}")
                    nc.vector.tensor_mul(zh, z, h_cur[:, cs])
                    nc.vector.tensor_add(hnew[:, cs], nz1, zh)

                    # ---- transposes for this column half ----
                    # chunks kc = s*4 + kk for kk in {2ch, 2ch+1}; pairs
                    # (s=0, s=1) go to distinct banks (tpa/tpb) and distinct
                    # row groups -> concurrent on the PE.
                    for kk in (2 * ch, 2 * ch + 1):
                        for s in range(2):
                            kc = s * NG + kk
                            tp = tpa if kc < 4 else tpb
                            nc.tensor.transpose(
                                tp[:, (kc % 4) * B:(kc % 4) * B + B],
                                hnew[s * B:(s + 1) * B, kk * 128:(kk + 1) * 128],
                                ident[s * B:(s + 1) * B, :],
                            )
                    # ---- store this column half of h' ----
                    for s in range(2):
                        nc.sync.dma_start(
                            out_d[:, t, s * 512 + ch * 256: s * 512 + ch * 256 + 256],
                            hnew[s * B:(s + 1) * B, cs],
                        )
                    # ---- pack transposed chunks into next-step hT tiles ----
                    # hTa cols: [kc0 | kc1 | kc4 | kc5], hTb: [kc2 | kc3 | kc6 | kc7]
                    a = ch  # ch=0 -> chunks {0,1,4,5} = hT tile 0
                    kk0 = 2 * ch
                    # one copy on ACT, one on DVE so they run in parallel
                    nc.scalar.copy(hT_new[a][:, 0:128], tpa[:, kk0 * B:(kk0 + 2) * B])
                    nc.vector.tensor_copy(hT_new[a][:, 128:256], tpb[:, kk0 * B:(kk0 + 2) * B])

                hT = hT_new
                h_cur = hnew

    nc.compile()
    return nc


def prepare_core_inputs(hidden, weight_ih, weight_hh, bias_ih, bias_hh):
    """Host-side preprocessing + batch sharding -> per-core input maps."""
    f32 = np.float32
    Wi = np.asarray(weight_ih, f32)
    Wh = np.asarray(weight_hh, f32)
    bi = np.asarray(bias_ih, f32)
    bh = np.asarray(bias_hh, f32)
    hidden = np.asarray(hidden, f32)

    # Wg[gi] : [H, H] with rows = gate output unit, cols = contraction k
    Wg = np.stack([
        Wi[0:H] + Wh[0:H],          # r
        Wi[H:2 * H] + Wh[H:2 * H],  # z
        Wi[2 * H:3 * H],            # in
        Wh[2 * H:3 * H],            # hn
    ])  # [4, H, H]
    # -> wdram[kc, p, (s*4+gi)*512 + j] = Wg[gi, s*512+j, kc*128+p]
    wdram = (
        Wg.reshape(NG, 2, 512, KC, 128)      # [gi, s, j, kc, p]
        .transpose(3, 4, 1, 0, 2)            # [kc, p, s, gi, j]
        .reshape(KC, 128, G)
        .astype(ml_dtypes.bfloat16)
    )

    bias_g = np.stack([
        bi[0:H] + bh[0:H],
        bi[H:2 * H] + bh[H:2 * H],
        bi[2 * H:3 * H],
        bh[2 * H:3 * H],
    ])  # [4, H]
    # bias rows: 0:64 slice-0 values, 64:128 slice-1; cols gi*512 + j
    bias_rows = [bias_g[:, s * 512:(s + 1) * 512].reshape(NG * 512) for s in range(2)]
    bias_t = np.concatenate([
        np.broadcast_to(bias_rows[0], (B, NG * 512)),
        np.broadcast_to(bias_rows[1], (B, NG * 512)),
    ], axis=0).astype(f32)
    bias_t = np.ascontiguousarray(bias_t)

    ident = np.ascontiguousarray(np.tile(np.eye(B, dtype=f32), (2, 1)))

    perm = [[0, 1, 4, 5], [2, 3, 6, 7]]
    in_maps = []
    for c in range(NCORES):
        h0 = np.ascontiguousarray(hidden[c * B:(c + 1) * B])  # [B, H]
        h0_2 = np.concatenate([h0[:, :512], h0[:, 512:]], axis=0)  # [128, 512]
        hh = h0.reshape(B, KC, 128)  # [b, kc, p]
        h0t = np.stack([hh[:, perm[0], :], hh[:, perm[1], :]])  # [a, b, idx, p]
        h0t = np.ascontiguousarray(h0t.transpose(0, 3, 2, 1)   # [a, p, idx, b]
                                   .reshape(2, 128, NG * B)).astype(ml_dtypes.bfloat16)
        in_maps.append({"w": wdram, "bias": bias_t, "h0": h0_2, "h0t": h0t,
                        "ident": ident})
    return in_maps


def _get_nc():
    if "nc" not in _NC_CACHE:
        _NC_CACHE["nc"] = build_nc(T)
    return _NC_CACHE["nc"]


def run(inputs: dict, trace: bool = False):
    """Returns (output [BATCH, T, H] fp32, BassKernelResults)."""
    in_maps = prepare_core_inputs(**inputs)
    res = run_bass_kernel_spmd(_get_nc(), in_maps, list(range(NCORES)),
                               trace=trace)
    out = np.concatenate([res.results[c]["out"] for c in range(NCORES)], axis=0)
    return out.astype(np.float32), res


def kernel(**inputs) -> np.ndarray:
    out, _ = run(inputs)
    return out
